# revision 10
# baseline (speedup 1.0000x reference)
"""GCN 2-layer (GCNConv -> relu -> GCNConv -> log_softmax) on 8 trn2 cores.

Math (equivalent to PyG GCNConv reference):
  h1s = dinv * (x @ W1)                         # gather table 1
  z1  = relu(dinv_d * seg_sum_{e->d}(h1s[src]) + b1)   # edges + self loop
  h2s = dinv * (z1 @ W2)                        # gather table 2
  out = log_softmax(dinv_d * seg_sum(h2s[src]) + b2)

Sharding: nodes 8 ways (12500/core, padded NL=12544). Each core aggregates
its dst shard. Edge slots grouped by (quarter-pass, src-window, dst-group)
padded to 128. dma_gather (int16, <=1024 rows/call, 4 SWDGE queues) reads
from 4 windows of 25088 rows of the table. Segment-sum: per-128-slot-tile
is_equal one-hot selector (DVE) + PE matmul accumulated in PSUM per dst
group. Tables exchanged between layers via AllGather.
"""
import numpy as np

import concourse.bass as bass
import concourse.bacc as bacc
import concourse.mybir as mybir
import concourse.tile as tile
from concourse.masks import make_identity
from concourse.bass_utils import run_bass_kernel_spmd

N = 100000
DIN, DH, DOUT = 128, 64, 40
NCORE = 8
NSH = 12500
NL = 12544              # 98 * 128
G = 98
TBL = NL * NCORE        # 100352
WROWS = TBL // 4        # 25088 (< 32767, int16-addressable)
NPASS = 17
PASSES = [list(range(6 * p, min(6 * p + 6, G))) for p in range(NPASS)]
NQ = 4

F32 = mybir.dt.float32
I16 = mybir.dt.int16


def _preprocess(edge_index):
    src = np.asarray(edge_index[0], dtype=np.int64)
    dst = np.asarray(edge_index[1], dtype=np.int64)
    deg = np.bincount(dst, minlength=N).astype(np.float64) + 1.0
    dinv = (1.0 / np.sqrt(deg)).astype(np.float32)

    allsrc = np.concatenate([src, np.arange(N, dtype=np.int64)])
    alldst = np.concatenate([dst, np.arange(N, dtype=np.int64)])
    trow = (allsrc // NSH) * NL + (allsrc % NSH)
    core = alldst // NSH
    dloc = alldst % NSH
    grp = dloc // 128
    q_of_grp = np.arange(G) // 6
    win = trow // WROWS
    l16 = (trow % WROWS).astype(np.int64)
    dl = (dloc % 128).astype(np.int64)

    # counts per (core, q, w, g)
    key = ((core * NPASS + q_of_grp[grp]) * 4 + win) * G + grp
    order = np.argsort(key, kind="stable")
    key_s = key[order]
    l16_s = l16[order]
    dl_s = dl[order]
    counts = np.bincount(key_s, minlength=NCORE * NPASS * 4 * G)
    counts = counts.reshape(NCORE, NPASS, 4, G)
    ntile = (counts.max(axis=0) + 127) // 128          # [4, 4, G]

    sched = []
    for q in range(NPASS):
        for w in range(4):
            tg = []
            for g in PASSES[q]:
                tg += [g] * int(ntile[q, w, g])
            sched.append((q, w, tg))
    tot_tiles = sum(len(s[2]) for s in sched)
    tot_slots = tot_tiles * 128

    # slot offset of each (q, w, g) cell in the flat layout
    cell_off = np.zeros((NPASS, 4, G), np.int64)
    pos = 0
    for q, w, tg in sched:
        for g in PASSES[q]:
            cell_off[q, w, g] = pos
            pos += int(ntile[q, w, g]) * 128

    idx16 = np.zeros((NCORE, tot_slots), np.int16)
    dstf = np.full((NCORE, tot_slots), 999.0, np.float32)
    bounds = np.searchsorted(key_s, np.arange(NCORE * NPASS * 4 * G + 1))
    for c in range(NCORE):
        for q in range(NPASS):
            for w in range(4):
                for g in PASSES[q]:
                    k = ((c * NPASS + q) * 4 + w) * G + g
                    a, b = bounds[k], bounds[k + 1]
                    if a == b:
                        continue
                    o = cell_off[q, w, g]
                    idx16[c, o:o + (b - a)] = l16_s[a:b]
                    dstf[c, o:o + (b - a)] = dl_s[a:b]

    # per-(q,w) segment -> gather calls of <=8 tiles
    calls = []
    for q, w, tg in sched:
        nt, cl = len(tg), []
        while nt > 0:
            k = min(8, nt)
            cl.append(k)
            nt -= k
        calls.append(cl)

    # last tile ordinal (within quarter) per group, for matmul stop flags
    last_tile = {}
    ti = 0
    for q, w, tg in sched:
        for g in tg:
            last_tile[g] = ti
            ti += 1
    return dinv, idx16, dstf, sched, calls, tot_tiles, last_tile


def _wrap_idx(flat):
    s = flat.shape[0]
    w = flat.reshape(s // 16, 16).T
    return np.tile(w, (8, 1)).copy()


def _build(sched, calls, tot_tiles, last_tile):
    ncalls = sum(len(cl) for cl in calls)
    idx_cols = tot_tiles * 8            # tot_slots/16

    nc = bacc.Bacc(None, target_bir_lowering=False, num_swdge_queues=NQ)
    xt = nc.declare_dram_parameter("xt", [128, NL], F32, isOutput=False)
    w1 = nc.declare_dram_parameter("w1", [DIN, DH], F32, isOutput=False)
    w2 = nc.declare_dram_parameter("w2", [DH, DOUT], F32, isOutput=False)
    b1r = nc.declare_dram_parameter("b1r", [128, DH], F32, isOutput=False)
    b2r = nc.declare_dram_parameter("b2r", [128, DOUT], F32, isOutput=False)
    dv = nc.declare_dram_parameter("dv", [128, G], F32, isOutput=False)
    idxd = nc.declare_dram_parameter("idxd", [128, idx_cols], I16, isOutput=False)
    dstd = nc.declare_dram_parameter("dstd", [128, tot_tiles], F32, isOutput=False)
    iod = nc.declare_dram_parameter("iod", [128, 128], F32, isOutput=False)
    out = nc.declare_dram_parameter("out", [NL, DOUT], F32, isOutput=True)

    t1l = nc.dram_tensor("t1l", [NL, DH], F32)
    t1f = nc.dram_tensor("t1f", [TBL, DH], F32)
    t2l = nc.dram_tensor("t2l", [NL, DH], F32)
    t2f = nc.dram_tensor("t2f", [TBL, DH], F32)

    with tile.TileContext(nc) as tc:
        with (
            tc.tile_pool(name="const", bufs=1) as cp,
            tc.tile_pool(name="xin", bufs=3) as xp,
            tc.tile_pool(name="gch", bufs=6) as gp,
            tc.tile_pool(name="sel", bufs=4) as sp,
            tc.tile_pool(name="epi", bufs=3) as ep,
            tc.tile_pool(name="zfm", bufs=1) as zp,
            tc.tile_pool(name="tf", bufs=1, space="PSUM") as tfp,
            tc.tile_pool(name="tr", bufs=1, space="PSUM") as trp,
            tc.tile_pool(name="agg", bufs=1, space="PSUM") as agp,
        ):
            w1t = cp.tile([DIN, DH], F32)
            nc.sync.dma_start(out=w1t[:], in_=w1[:, :])
            w2t = cp.tile([DH, DOUT], F32)
            nc.sync.dma_start(out=w2t[:], in_=w2[:, :])
            b1t = cp.tile([128, DH], F32)
            nc.sync.dma_start(out=b1t[:], in_=b1r[:, :])
            b2t = cp.tile([128, DOUT], F32)
            nc.sync.dma_start(out=b2t[:], in_=b2r[:, :])
            dvt = cp.tile([128, G], F32)
            nc.sync.dma_start(out=dvt[:], in_=dv[:, :])
            iot = cp.tile([128, 128], F32)
            nc.sync.dma_start(out=iot[:], in_=iod[:, :])
            idxt = cp.tile([128, idx_cols], I16)
            nc.sync.dma_start(out=idxt[:], in_=idxd[:, :])
            dstt = cp.tile([128, tot_tiles], F32)
            nc.sync.dma_start(out=dstt[:], in_=dstd[:, :])
            idn = cp.tile([128, 128], F32)
            make_identity(nc, idn[:])
            zcol = cp.tile([128, DH - DOUT], F32)
            nc.gpsimd.memset(zcol[:], 0.0)
            z1t = zp.tile([DH, NL], F32)

            # ---- Phase A: local L1 table rows ----
            for g in range(G):
                xc = xp.tile([128, 128], F32)
                nc.sync.dma_start(out=xc[:], in_=xt[:, g * 128:(g + 1) * 128])
                p1 = tfp.tile([128, 128], F32, tag="p1")
                nc.tensor.matmul(out=p1[:DH, :], lhsT=w1t[:], rhs=xc[:],
                                 start=True, stop=True)
                c1 = ep.tile([128, 128], F32, tag="c1")
                nc.vector.tensor_copy(out=c1[:DH, :], in_=p1[:DH, :])
                p2 = trp.tile([128, DH], F32, tag="p2")
                nc.tensor.transpose(out=p2[:, :DH], in_=c1[:DH, :],
                                    identity=idn[:DH, :DH])
                r = ep.tile([128, DH], F32, tag="r")
                nc.vector.tensor_scalar_mul(out=r[:], in0=p2[:],
                                            scalar1=dvt[:, g:g + 1])
                nc.sync.dma_start(out=t1l[g * 128:(g + 1) * 128, :], in_=r[:])

            nc.gpsimd.collective_compute(
                "AllGather", mybir.AluOpType.bypass,
                replica_groups=[list(range(NCORE))],
                ins=[t1l[:, :].opt()], outs=[t1f[:, :].opt()])

            def aggregate(table, width, epilogue):
                tview = table.rearrange("(w r) d -> w r d", w=4)
                col_off = 0
                tile_i = 0
                qi_prev = -1
                agg_t = None
                started = set()
                for (q, w, tg), cl in zip(sched, calls):
                    if q != qi_prev:
                        if agg_t is not None:
                            for j, g in enumerate(PASSES[qi_prev]):
                                epilogue(g, agg_t, j)
                        agg_t = [agp.tile([128, DH], F32, tag=f"agg{j}",
                                          name=f"agg{j}")
                                 for j in range(len(PASSES[q]))]
                        started = set()
                        qi_prev = q
                    ti_seg = 0
                    for ncols in cl:
                        ni = ncols * 128
                        gch = gp.tile([128, 8, DH], F32)
                        nc.gpsimd.dma_gather(
                            out_ap=gch[:, :ncols, :], in_ap=tview[w, :, :],
                            idxs_ap=idxt[:, col_off:col_off + ni // 16],
                            num_idxs=ni, num_idxs_reg=ni, elem_size=DH,
                            queue_num=(col_off // 64) % NQ)
                        col_off += ni // 16
                        for j in range(ncols):
                            g = tg[ti_seg]
                            go = g - PASSES[q][0]
                            sel = sp.tile([128, 128], F32)
                            nc.vector.tensor_scalar(
                                out=sel[:], in0=iot[:],
                                scalar1=dstt[:, tile_i:tile_i + 1],
                                scalar2=None, op0=mybir.AluOpType.is_equal)
                            first = g not in started
                            started.add(g)
                            nc.tensor.matmul(
                                out=agg_t[go][:, :width],
                                lhsT=sel[:], rhs=gch[:, j, :width],
                                start=first, stop=(tile_i == last_tile[g]))
                            ti_seg += 1
                            tile_i += 1
                for j, g in enumerate(PASSES[qi_prev]):
                    epilogue(g, agg_t, j)

            # ---- L1 epilogue ----
            def epi1(g, agg_t, j):
                psl = agg_t[j][:, :DH]
                u = ep.tile([128, DH], F32, tag="u")
                nc.vector.tensor_scalar_mul(out=u[:], in0=psl,
                                            scalar1=dvt[:, g:g + 1])
                v = ep.tile([128, DH], F32, tag="v")
                nc.vector.tensor_add(out=v[:], in0=u[:], in1=b1t[:])
                z = ep.tile([128, DH], F32, tag="z")
                nc.scalar.activation(out=z[:], in_=v[:],
                                     func=mybir.ActivationFunctionType.Relu)
                pz = tfp.tile([128, 128], F32, tag="p1")
                nc.tensor.transpose(out=pz[:DH, :], in_=z[:], identity=idn[:])
                nc.vector.tensor_copy(out=z1t[:, g * 128:(g + 1) * 128],
                                      in_=pz[:DH, :])

            aggregate(t1f, DH, epi1)

            # ---- Phase C: local L2 table rows (cols 40:64 zero) ----
            for g in range(G):
                p1 = tfp.tile([128, 128], F32, tag="p1")
                nc.tensor.matmul(out=p1[:DOUT, :], lhsT=w2t[:],
                                 rhs=z1t[:, g * 128:(g + 1) * 128],
                                 start=True, stop=True)
                c1 = ep.tile([128, 128], F32, tag="c1")
                nc.vector.tensor_copy(out=c1[:DOUT, :], in_=p1[:DOUT, :])
                p2 = trp.tile([128, DH], F32, tag="p2")
                nc.tensor.transpose(out=p2[:, :DOUT], in_=c1[:DOUT, :],
                                    identity=idn[:DOUT, :DOUT])
                r = ep.tile([128, DH], F32, tag="r")
                nc.vector.tensor_copy(out=r[:, DOUT:], in_=zcol[:])
                nc.vector.tensor_scalar_mul(out=r[:, :DOUT], in0=p2[:, :DOUT],
                                            scalar1=dvt[:, g:g + 1])
                nc.sync.dma_start(out=t2l[g * 128:(g + 1) * 128, :], in_=r[:])

            nc.gpsimd.collective_compute(
                "AllGather", mybir.AluOpType.bypass,
                replica_groups=[list(range(NCORE))],
                ins=[t2l[:, :].opt()], outs=[t2f[:, :].opt()])

            # ---- L2 epilogue: log_softmax ----
            def epi2(g, agg_t, j):
                psl = agg_t[j][:, :DOUT]
                u = ep.tile([128, DOUT], F32, tag="u2")
                nc.vector.tensor_scalar_mul(out=u[:], in0=psl,
                                            scalar1=dvt[:, g:g + 1])
                v = ep.tile([128, DOUT], F32, tag="v2")
                nc.vector.tensor_add(out=v[:], in0=u[:], in1=b2t[:])
                m = ep.tile([128, 1], F32, tag="m")
                nc.vector.reduce_max(out=m[:], in_=v[:],
                                     axis=mybir.AxisListType.X)
                w_ = ep.tile([128, DOUT], F32, tag="w")
                nc.vector.tensor_scalar(out=w_[:], in0=v[:], scalar1=m[:],
                                        scalar2=None,
                                        op0=mybir.AluOpType.subtract)
                e = ep.tile([128, DOUT], F32, tag="e")
                nc.scalar.activation(out=e[:], in_=w_[:],
                                     func=mybir.ActivationFunctionType.Exp)
                sm = ep.tile([128, 1], F32, tag="sm")
                nc.vector.reduce_sum(out=sm[:], in_=e[:],
                                     axis=mybir.AxisListType.X)
                ls = ep.tile([128, 1], F32, tag="ls")
                nc.scalar.activation(out=ls[:], in_=sm[:],
                                     func=mybir.ActivationFunctionType.Ln)
                o = ep.tile([128, DOUT], F32, tag="o")
                nc.vector.tensor_scalar(out=o[:], in0=w_[:], scalar1=ls[:],
                                        scalar2=None,
                                        op0=mybir.AluOpType.subtract)
                nc.sync.dma_start(out=out[g * 128:(g + 1) * 128, :], in_=o[:])

            aggregate(t2f, DOUT, epi2)

    nc.finalize()
    return nc


_CACHE = {}


def kernel(x, W1, b1, W2, b2, edge_index):
    x = np.asarray(x, np.float32)
    W1 = np.asarray(W1, np.float32)
    b1 = np.asarray(b1, np.float32)
    W2 = np.asarray(W2, np.float32)
    b2 = np.asarray(b2, np.float32)
    ckey = hash(np.asarray(edge_index).tobytes())
    if ckey in _CACHE:
        dinv, idx16, dstf, sched, calls, tot_tiles, last_tile, nc = _CACHE[ckey]
    else:
        dinv, idx16, dstf, sched, calls, tot_tiles, last_tile = _preprocess(
            edge_index)
        nc = _build(sched, calls, tot_tiles, last_tile)
        _CACHE[ckey] = (dinv, idx16, dstf, sched, calls, tot_tiles,
                        last_tile, nc)

    iota = np.tile(np.arange(128, dtype=np.float32), (128, 1))
    b1rep = np.tile(b1, (128, 1)).astype(np.float32)
    b2rep = np.tile(b2, (128, 1)).astype(np.float32)

    in_maps = []
    for c in range(NCORE):
        xtc = np.zeros((128, NL), np.float32)
        xtc[:, :NSH] = x[c * NSH:(c + 1) * NSH, :].T
        dfull = np.zeros(NL, np.float32)
        dfull[:NSH] = dinv[c * NSH:(c + 1) * NSH]
        in_maps.append({
            "xt": xtc, "w1": W1, "w2": W2, "b1r": b1rep, "b2r": b2rep,
            "dv": np.ascontiguousarray(dfull.reshape(G, 128).T),
            "idxd": _wrap_idx(idx16[c]),
            "dstd": np.ascontiguousarray(dstf[c].reshape(tot_tiles, 128).T),
            "iod": iota,
        })
    res = run_bass_kernel_spmd(nc, in_maps, list(range(NCORE)))
    outv = np.empty((N, DOUT), np.float32)
    for c in range(NCORE):
        outv[c * NSH:(c + 1) * NSH] = res.results[c]["out"][:NSH]
    return outv
